# revision 14
# baseline (speedup 1.0000x reference)
"""Causal self-attention (B=4, T=2048, C=768, H=12) on 8 Trainium2 cores.

Sharding (Megatron-style hybrid): core c handles batch b = c//2 and head-group
g = c%2 (6 heads, 384 channels). Q/K/V weights are column-split per group, the
output projection row-split; each core emits a partial y that the host sums
over the two groups of a batch.

Per-core kernel, software-pipelined over 512-token chunks (qr):
  proj(qr):  Q^T/K^T chunks via matmul + DVE bias-cast to bf16;
             V chunk + bias -> error-compensated fp8 pair (hi, lo) laid out
             per head as [V|1] / [1|V] column blocks (the ones column block
             makes PV emit the softmax denominator l on 64 partitions).
  attn(qr):  S^T tile = matmul(lhsT=K^T, rhs=Q^T) in bf16, causal mask added
             *pre-exp* by a (-2^30 * I) @ triU matmul into the same PSUM
             accumulation group; ScalarE exp (scale 1/8, bias -1) emits P
             directly in fp8e4; PV = fp8 DoubleRow matmul with slices
             (V_hi, V_lo) against P duplicated via a 0-stride AP -- full
             error-compensated V at half cost. Diagonal-block column ranges
             are trimmed everywhere (S, exp, PV).
  norm(qr):  per head, DVE reciprocal of the l rows (PSUM) and one
             tensor_tensor multiply -> O^T bf16 (no partition-shift DMAs).
  y(qr):     y tile = O^T-chunks @ Wp accumulated in PSUM, DMA'd to HBM
             straight from PSUM.

ScalarE runs only the exps (the softmax wall); everything element-wise
else is on DVE; masks ride the TensorE; Pool only triggers DMAs.
"""

import numpy as np
import ml_dtypes

import concourse.bass as bass
import concourse.tile as tile
from concourse import library_config
import concourse.mybir as mybir
from concourse import bacc
from concourse.bass_utils import run_bass_kernel_spmd

F32 = mybir.dt.float32
BF16 = mybir.dt.bfloat16
FP8 = mybir.dt.float8e4
BF = ml_dtypes.bfloat16

B, T, C, H, D = 4, 2048, 768, 12, 64
G = 2                    # head groups (tensor-parallel degree)
HG = H // G              # heads per group = 6
CG = C // G              # channels per group = 384
HP = HG // 2             # head pairs per group = 3
KC = C // 128            # contraction chunks over C = 6
QR = T // 512            # 512-wide q ranges = 4
TT = T // 128            # 128-wide token tiles = 16
NCORES = 8
SCALE = 1.0 / np.sqrt(D)
EXP_BIAS = -1.0          # exp(s/8 - 1): keeps P < 240 (fp8e4 max) w/ margin

_nc_cache = {}


def _build_nc(reps=1, loop=False):
    nc = bacc.Bacc(None, target_bir_lowering=False, debug=False,
                   num_devices=NCORES, name="csa")
    if loop:
        ni_d = nc.dram_tensor("niter", [1, 1], mybir.dt.int32, kind="ExternalInput")

    xt_d = nc.dram_tensor("xt", [C, T], BF16, kind="ExternalInput")
    wq_d = nc.dram_tensor("wq", [C, CG], BF16, kind="ExternalInput")
    wk_d = nc.dram_tensor("wk", [C, CG], BF16, kind="ExternalInput")
    wv_d = nc.dram_tensor("wv", [C, CG], BF16, kind="ExternalInput")
    wp_d = nc.dram_tensor("wp", [CG, C], BF16, kind="ExternalInput")
    tri_d = nc.dram_tensor("triu", [128, 128], BF16, kind="ExternalInput")
    ngi_d = nc.dram_tensor("negi", [128, 128], BF16, kind="ExternalInput")
    bqk_d = nc.dram_tensor("bqk", [128, 2 * HP], F32, kind="ExternalInput")
    bv_d = nc.dram_tensor("bv_bc", [128, CG], F32, kind="ExternalInput")
    y_d = nc.dram_tensor("y", [T, C], F32, kind="ExternalOutput")

    with tile.TileContext(nc) as tc:
        with (
            tc.tile_pool(name="persist", bufs=1) as pers,
            tc.tile_pool(name="work", bufs=6) as work,
            tc.tile_pool(name="nrm", bufs=4) as nrm,
            tc.tile_pool(name="ps", bufs=2, space="PSUM") as ps,
            tc.tile_pool(name="po", bufs=1, space="PSUM") as po,
            tc.tile_pool(name="pj", bufs=1, space="PSUM") as pj,
        ):
            # ---- persistent SBUF tensors ----
            xt_t = pers.tile([128, KC, T], BF16)
            wq_t = pers.tile([128, KC, CG], BF16)
            wk_t = pers.tile([128, KC, CG], BF16)
            wv_t = pers.tile([128, KC, CG], BF16)
            wp_t = pers.tile([128, HP, C], BF16)
            tri_t = pers.tile([128, 128], BF16)
            ngi_t = pers.tile([128, 128], BF16)
            bqk_t = pers.tile([128, 2 * HP], F32)
            bv_t = pers.tile([128, CG], F32)
            eb_t = pers.tile([128, 1], F32)
            qt_ts = [pers.tile([128, T], BF16, tag=f"qt{i}", name=f"qt{i}") for i in range(HP)]
            kt_ts = [pers.tile([128, T], BF16, tag=f"kt{i}", name=f"kt{i}") for i in range(HP)]
            # per key-tile: [partition=key, head, slice(hi/lo), col] fp8
            # slice0 = [V_hi | ones], slice1 = [V_lo | zeros]:
            # PV DoubleRow emits O rows 0:64 and the softmax denominator l row 64
            v8_ts = [pers.tile([128, HG, 2, 128], FP8, tag=f"v8{i}", name=f"v8{i}")
                     for i in range(TT)]
            on_ts = [pers.tile([128, T], BF16, tag=f"on{i}", name=f"on{i}") for i in range(HP)]

            xt_r = xt_d[:].rearrange("(kc p) t -> p kc t", p=128)
            for kc in range(KC):
                nc.gpsimd.dma_start(xt_t[:, kc, :], xt_r[:, kc, :])
            nc.gpsimd.dma_start(wq_t[:], wq_d[:].rearrange("(kc p) f -> p kc f", p=128))
            nc.gpsimd.dma_start(wk_t[:], wk_d[:].rearrange("(kc p) f -> p kc f", p=128))
            nc.gpsimd.dma_start(wv_t[:], wv_d[:].rearrange("(kc p) f -> p kc f", p=128))
            nc.gpsimd.dma_start(wp_t[:], wp_d[:].rearrange("(hp p) f -> p hp f", p=128))
            nc.gpsimd.dma_start(tri_t[:], tri_d[:])
            nc.gpsimd.dma_start(ngi_t[:], ngi_d[:])
            nc.gpsimd.dma_start(bqk_t[:], bqk_d[:])
            nc.gpsimd.dma_start(bv_t[:], bv_d[:])
            nc.vector.memset(eb_t[:], EXP_BIAS)
            for tt in range(TT):
                nc.vector.memset(v8_ts[tt][:, :, 0, D:128], 1.0)
                nc.vector.memset(v8_ts[tt][:, :, 1, D:128], 0.0)
            nc.gpsimd.load_library(library_config.attn)

            if loop:
                ni_t = pers.tile([1, 1], mybir.dt.int32)
                nc.gpsimd.dma_start(ni_t[:], ni_d[:])
                ni_reg = nc.values_load(ni_t[0:1, 0:1].to_broadcast((1, 1)))
                loop_cm = tc.For_i(0, ni_reg, 1)
                loop_cm.__enter__()

            for _rep in range(reps):
                for qr in range(QR):
                    q0 = qr * 512
                    # ---- proj chunk qr: V tiles then Q|K ----
                    for tt in range(4 * qr, 4 * qr + 4):
                        v_ps = pj.tile([128, 1024], F32, tag="pj")
                        for kc in range(KC):
                            nc.tensor.matmul(
                                v_ps[:, 0:CG], xt_t[:, kc, tt * 128:(tt + 1) * 128],
                                wv_t[:, kc, :],
                                start=(kc == 0), stop=(kc == KC - 1))
                        vb_t = work.tile([128, CG], F32, tag="vb")
                        nc.vector.tensor_tensor(vb_t[:], v_ps[:, 0:CG], bv_t[:],
                                                mybir.AluOpType.add)
                        vb3 = vb_t[:].rearrange("p (h d) -> p h d", h=HG)
                        v8 = v8_ts[tt]
                        nc.vector.tensor_copy(v8[:, :, 0, 0:D], vb3[:])
                        nc.vector.tensor_tensor(v8[:, :, 1, 0:D], vb3[:],
                                                v8[:, :, 0, 0:D],
                                                mybir.AluOpType.subtract)
                    for fc in range(HP):
                        qk_ps = pj.tile([128, 1024], F32, tag="pj")
                        for kc in range(KC):
                            nc.tensor.matmul(
                                qk_ps[:, 0:512], wq_t[:, kc, fc * 128:(fc + 1) * 128],
                                xt_t[:, kc, q0:q0 + 512],
                                start=(kc == 0), stop=(kc == KC - 1))
                        for kc in range(KC):
                            nc.tensor.matmul(
                                qk_ps[:, 512:1024], wk_t[:, kc, fc * 128:(fc + 1) * 128],
                                xt_t[:, kc, q0:q0 + 512],
                                start=(kc == 0), stop=(kc == KC - 1))
                        nc.vector.tensor_tensor(
                            qt_ts[fc][:, q0:q0 + 512], qk_ps[:, 0:512],
                            bqk_t[:, fc:fc + 1].to_broadcast((128, 512)),
                            mybir.AluOpType.add)
                        nc.vector.tensor_tensor(
                            kt_ts[fc][:, q0:q0 + 512], qk_ps[:, 512:1024],
                            bqk_t[:, HP + fc:HP + fc + 1].to_broadcast((128, 512)),
                            mybir.AluOpType.add)

                    # ---- attention chunk qr ----
                    nki = 4 * qr + 4
                    for hp in range(HP):
                        o_ps = po.tile([128, 1024], F32, tag="o")
                        for ki in range(nki):
                            j = ki - 4 * qr
                            s_ps = ps.tile([128, 1024], F32, tag="s")
                            for h in range(2):
                                base = h * 512
                                kA = kt_ts[hp][64 * h:64 * h + 64, ki * 128:(ki + 1) * 128]
                                if j < 0:
                                    nc.tensor.matmul(
                                        s_ps[:, base:base + 512], kA,
                                        qt_ts[hp][64 * h:64 * h + 64, q0:q0 + 512],
                                        start=True, stop=True)
                                else:
                                    c0 = 128 * j
                                    nc.tensor.matmul(
                                        s_ps[:, base + c0:base + c0 + 128], kA,
                                        qt_ts[hp][64 * h:64 * h + 64, q0 + c0:q0 + c0 + 128],
                                        start=True, stop=False)
                                    nc.tensor.matmul(
                                        s_ps[:, base + c0:base + c0 + 128],
                                        ngi_t[:], tri_t[:],
                                        start=False, stop=True)
                                    if j < 3:
                                        nc.tensor.matmul(
                                            s_ps[:, base + c0 + 128:base + 512], kA,
                                            qt_ts[hp][64 * h:64 * h + 64,
                                                      q0 + c0 + 128:q0 + 512],
                                            start=True, stop=True)
                            c0 = 128 * j if j > 0 else 0
                            p8_t = work.tile([128, 2, 512], FP8, tag="p8")
                            p3 = p8_t[:]
                            s3 = s_ps[:].rearrange("p (two q) -> p two q", two=2)
                            nc.scalar.activation(
                                p3[:, :, c0:], s3[:, :, c0:],
                                mybir.ActivationFunctionType.Exp,
                                scale=SCALE, bias=eb_t[:, 0:1])
                            for h in range(2):
                                nc.tensor.matmul(
                                    o_ps[:, h * 512 + c0:h * 512 + 512],
                                    v8_ts[ki][:, 2 * hp + h, :, :],
                                    p8_t[:, h, None, c0:].to_broadcast(
                                        (128, 2, 512 - c0)),
                                    start=(ki == 0), stop=(ki == nki - 1),
                                    perf_mode=mybir.MatmulPerfMode.DoubleRow,
                                    skip_group_check=True)
                        # softmax denominators: recip of PSUM row 64, bcast, mul
                        l_row = nrm.tile([1, 1024], F32, tag="l")
                        nc.vector.tensor_copy(l_row[:], o_ps[D:D + 1, :])
                        r_row = nrm.tile([1, 1024], F32, tag="r")
                        nc.vector.reciprocal_approx_fast(out=r_row[:], in_=l_row[:])
                        r_bc = nrm.tile([64, 1024], F32, tag="rb")
                        nc.gpsimd.partition_broadcast(r_bc[:], r_row[:], channels=64)
                        nc.vector.tensor_tensor(
                            on_ts[hp][0:64, q0:q0 + 512],
                            o_ps[0:64, 0:512], r_bc[:, 0:512],
                            mybir.AluOpType.mult)
                        ob_t = nrm.tile([64, 512], BF16, tag="ob")
                        nc.vector.tensor_tensor(
                            ob_t[:], o_ps[0:64, 512:1024], r_bc[:, 512:1024],
                            mybir.AluOpType.mult)
                        nc.sync.dma_start(
                            on_ts[hp][64:128, q0:q0 + 512], ob_t[:])

                    # ---- output projection chunk qr ----
                    for tt in range(4 * qr, 4 * qr + 4):
                        y_ps = pj.tile([128, 1024], F32, tag="pj")
                        for hp in range(HP):
                            lhsT = on_ts[hp][:, tt * 128:(tt + 1) * 128]
                            nc.tensor.matmul(y_ps[:, 0:512], lhsT, wp_t[:, hp, 0:512],
                                             start=(hp == 0), stop=(hp == HP - 1))
                            nc.tensor.matmul(y_ps[:, 512:768], lhsT, wp_t[:, hp, 512:768],
                                             start=(hp == 0), stop=(hp == HP - 1))
                        y_sb = work.tile([128, C], F32, tag="y")
                        nc.vector.tensor_copy(y_sb[:], y_ps[:, 0:768])
                        nc.sync.dma_start(y_d[tt * 128:(tt + 1) * 128, :], y_sb[:])

            if loop:
                loop_cm.__exit__(None, None, None)

    nc.finalize()
    return nc


def _causal_masks():
    f = np.arange(128)[None, :]
    k = np.arange(128)[:, None]
    triu = (k > f).astype(BF)                       # strictly-below-diag keys masked
    negi = (-(2.0 ** 30) * np.eye(128)).astype(BF)  # mask scale rides TensorE
    return triu, negi


def kernel(x, Wq, bq, Wk, bk, Wv, bv, Wp, bp):
    x, Wq, bq, Wk, bk, Wv, bv, Wp, bp = (
        np.asarray(a, dtype=np.float32)
        for a in (x, Wq, bq, Wk, bk, Wv, bv, Wp, bp))

    if "nc" not in _nc_cache:
        _nc_cache["nc"] = _build_nc()
    nc = _nc_cache["nc"]

    triu, negi = _causal_masks()
    in_maps = []
    for c in range(NCORES):
        b, g = c // 2, c % 2
        sl = slice(g * CG, (g + 1) * CG)
        bqk = np.concatenate([bq[sl].reshape(HP, 128).T,
                              bk[sl].reshape(HP, 128).T], axis=1)
        in_maps.append({
            "xt": np.ascontiguousarray(x[b].T).astype(BF),
            "wq": np.ascontiguousarray(Wq[:, sl]).astype(BF),
            "wk": np.ascontiguousarray(Wk[:, sl]).astype(BF),
            "wv": np.ascontiguousarray(Wv[:, sl]).astype(BF),
            "wp": np.ascontiguousarray(Wp[sl, :]).astype(BF),
            "triu": triu,
            "negi": negi,
            "bqk": np.ascontiguousarray(bqk).astype(np.float32),
            "bv_bc": np.tile(bv[sl][None, :], (128, 1)).astype(np.float32),
        })

    res = run_bass_kernel_spmd(nc, in_maps, core_ids=list(range(NCORES)))
    out = np.empty((B, T, C), np.float32)
    for b in range(B):
        out[b] = res.results[2 * b]["y"] + res.results[2 * b + 1]["y"] + bp
    return out
